# revision 31
# baseline (speedup 1.0000x reference)
"""Multi-head attention (B=2, N=4096, C=512, H=8, d=64) on 8 Trainium2 NeuronCores.

Sharding: core c handles batch b = c//4 and heads {2*(c%4), 2*(c%4)+1}.
Each core computes its 2 heads' attention plus a partial output projection
(contraction over its 128 rows of W_proj); the host gather sums the 4
partials per batch and adds the bias.

On-device dataflow (transposed-scores formulation, no on-chip transposes):
  qT/kT [128=2*64 d-dims, 4096]  = W.T @ x.T      (x.T supplied by host)
  v_aug [128 n-chunk, 32*(65+65)] = x @ Wv with a ones column per head
  S^T[kidx, q] = kT.T_chunk @ qT  (two heads ride row-groups 0-1 / 2-3
                                   of the PE array concurrently, K=64 each)
  E = exp(S^T / 8)   -- ScalarE activation, one [128,1024] instruction per
                        step; this engine paces the steady state (1128ns/step).
                        A custom fused DVE exp (e^(s/8) = p(a*s)^32, two uOp
                        passes, ~3e-4 rel) exists behind NO_DVE_EXP=0/DVE_N,
                        but offloading columns to it consistently measures
                        ~40-70us SLOWER end to end (DVE-queue latency bursts
                        delay est -> AV stalls the PE, plus higher sustained
                        power triggers utilization throttling).
  [out_unnorm^T; den] = v_aug.T @ E   (ones column makes row 64 the softmax
                                       denominator -- no extra pass)
  out^T = out_unnorm^T * (1/den)  (reciprocal_approx_fast + K=2 broadcast
                                   matmul + DVE multiply)
  partial = out^T.T @ W_proj_slice   (K=128 packed contraction; AV runs two
                                      steps behind scores so the PE never
                                      waits on exp)
"""

import sys
import types

for _p in ("/opt/trn_rl_repo",):
    if _p not in sys.path:
        sys.path.insert(0, _p)

import numpy as np
import ml_dtypes
from contextlib import ExitStack

# antenv.axon_hooks shim: lets run_bass_kernel_spmd find the NTFF profiling
# hook when BASS_TRACE=1 (the agent image's antenv lacks this module).
import antenv  # noqa: F401

if "antenv.axon_hooks" not in sys.modules:
    _m = types.ModuleType("antenv.axon_hooks")
    _m._hook = None

    def _set_hook(h):
        _m._hook = h

    def _get_hook():
        return _m._hook

    _m.set_axon_ntff_profile_hook = _set_hook
    _m.get_axon_ntff_profile_hook = _get_hook
    sys.modules["antenv.axon_hooks"] = _m
    try:
        from trn_agent_boot.trn_boot import _ntff_profile_via_ctypes

        hook = _ntff_profile_via_ctypes("/opt/axon/libaxon_pjrt.so")
        if hook is not None:
            _set_hook(hook)
    except Exception:
        pass

import concourse.bass as bass  # noqa: E402
import concourse.tile as tile  # noqa: E402
from concourse.tile import add_dep_helper  # noqa: E402
from concourse import mybir, bacc  # noqa: E402
from concourse import bass_utils  # noqa: E402

# No bucket storage in this container; artifacts stay local.
bass_utils.upload_artifacts = lambda tmpdir: f"local://{tmpdir}"
# (walrus --enable-ldw-opt=true was tried and rejects this kernel:
# "InstLdweights is not compatible with LDW optimization")

# ---------------- custom DVE exp ops ----------------
from concourse import dve_ops  # noqa: E402
from concourse.dve_spec import Spec, Src0, C0, C1, C2, One, lower  # noqa: E402
from concourse.dve_spec import _has_src1 as has_src1  # noqa: E402
from concourse.dve_uop import DveOpSpec  # noqa: E402

EXP_A = 0.002765511489989489   # z = a*s; e^(s/8) = (1 + c1 z + z^2 + c3 z^3)^32
EXP_C1 = 1.4125294899368124
EXP_C3 = 0.46696463288916246


def _register_op(name, spec):
    for op in dve_ops.OPS:
        if op.name == name:
            return op
    row = dve_ops._CUSTOM_DVE_ROW_BASE + len(dve_ops.OPS)
    uops = lower(spec, ver="v3")
    sha = DveOpSpec(
        name=name, opcode=row, uops=uops, rd1_en=has_src1(spec)
    ).sha("v3")
    op = dve_ops.DveOp(name, spec, subdim=False, uops_sha={"v3": sha})
    dve_ops.OPS.append(op)
    dve_ops.CUSTOM_DVE_SPECS[name] = spec
    dve_ops._SUB_OPCODE_FOR_NAME[name] = row
    return op


def _p1_ref(in0, in1, s0, s1, imm2):
    z = (in0.astype(np.float32) * np.float32(s0)).astype(np.float32)
    q = (z * z).astype(np.float32)
    return (((np.float32(s1) + z) + np.float32(imm2) * q) * z).astype(np.float32)


def _p2_ref(in0, in1, s0, s1, imm2):
    p = in0.astype(np.float32) + np.float32(1.0)
    for _ in range(5):
        p = (p * p).astype(np.float32)
    return p


_z = Src0 * C0
_q = _z * _z
EXP32_P1 = _register_op(
    "ANT_EXP32_P1", Spec(body=((C1 + _z) + C2 * _q) * _z, reference=_p1_ref)
)
_p = Src0 + One
_r1 = _p * _p
_r2 = _r1 * _r1
_r3 = _r2 * _r2
_r4 = _r3 * _r3
EXP32_P2 = _register_op(
    "ANT_EXP32_P2", Spec(body=_r4 * _r4, reference=_p2_ref)
)

B, N, C = 2, 4096, 512
H, D = 8, 64
N_CORES = 8
SCALE = D ** -0.5

BF16 = mybir.dt.bfloat16
F32 = mybir.dt.float32
AF = mybir.ActivationFunctionType
BFNP = ml_dtypes.bfloat16

NI = N // 128   # 32 kidx / n chunks
VW = 2 * (D + 1)  # 130: per-n-chunk vaug block (2 heads x (64 v + 1 ones))


import os as _os

# ScalarE-only exp is the default: offloading exp columns to the DVE
# raises sustained power and triggers hardware utilization throttling
# that costs more than the offload saves (measured: blocks degrade from
# 1128ns/step to 1300+ with 60-90us of throttle-active time).
_NO_DVE_EXP = _os.environ.get("NO_DVE_EXP", "1") == "1"
_NO_POOL = _os.environ.get("NO_POOL", "0") == "1"


_DVE_N = int(_os.environ.get("DVE_N", "0"))
# AV lag 8 (vs 2): deeper decoupling of the AV accumulation from the exp
# producer absorbs ScalarE latency bursts so the ACT stream stays gapless;
# measured 298973ns (lag 8) vs 299762ns (lag 6) vs ~308000ns (lag 4) vs
# 333584ns (lag 2) -- same numerics, emission order only. Needs expp
# bufs >= lag + 2 and the lag-aware norm/proj slots below.
_AV_LAG = int(_os.environ.get("AV_LAG", "8"))


def _cs_for(gs):
    """ScalarE exp column count for step gs; DVE does [cs:1024]."""
    if _NO_DVE_EXP or _DVE_N == 0:
        return 1024
    if gs < 64:
        return max(928, 1024 - _DVE_N)
    return 1024 - _DVE_N


def build_nc():
    nc = bacc.Bacc("TRN2", target_bir_lowering=False, debug=False)

    xt = nc.dram_tensor("xt", [4, 128, N], BF16, kind="ExternalInput").ap()
    wq = nc.dram_tensor("wq", [128, 512], BF16, kind="ExternalInput").ap()
    wk = nc.dram_tensor("wk", [128, 512], BF16, kind="ExternalInput").ap()
    wv = nc.dram_tensor("wv", [128, 512], BF16, kind="ExternalInput").ap()
    wp = nc.dram_tensor("wp", [128, 512], BF16, kind="ExternalInput").ap()
    out = nc.dram_tensor("out", [N, C], BF16, kind="ExternalOutput").ap()

    with tile.TileContext(nc) as tc:
        with ExitStack() as ctx:
            const = ctx.enter_context(tc.tile_pool(name="const", bufs=1))
            sb = ctx.enter_context(tc.tile_pool(name="sb", bufs=1))
            expp = ctx.enter_context(tc.tile_pool(name="expp", bufs=10))
            midp = ctx.enter_context(tc.tile_pool(name="midp", bufs=3))
            invp = ctx.enter_context(tc.tile_pool(name="invp", bufs=2))
            outp = ctx.enter_context(tc.tile_pool(name="outp", bufs=3))

            # twq/twk ride the HW DMA queues (sync/scalar): the gpsimd
            # software DGE delivers them ~2.5us later (1300ns init + queue)
            # and the prologue's first matmul needs twq.
            twq = const.tile([128, 512], BF16)
            nc.sync.dma_start(twq[:], wq[:])
            twk = const.tile([128, 512], BF16)
            nc.scalar.dma_start(twk[:], wk[:])
            twv = const.tile([128, 512], BF16)
            twp0 = const.tile([64, 512], BF16)
            twp1 = const.tile([64, 512], BF16)
            tones = const.tile([1, 128], BF16)
            nc.gpsimd.memset(tones[:], 1.0)

            qT = sb.tile([128, N], BF16)
            kT = sb.tile([128, N], BF16)
            vaug = sb.tile([128, NI * VW], BF16)
            nc.gpsimd.memset(vaug[:], 1.0)
            outT0 = sb.tile([64, N], BF16)
            outT1 = sb.tile([64, N], BF16)
            outTs = (outT0, outT1)

            xtp = ctx.enter_context(tc.tile_pool(name="xtp", bufs=1))
            psS = ctx.enter_context(tc.tile_pool(name="psS", bufs=2, space="PSUM"))
            psAV = ctx.enter_context(tc.tile_pool(name="psAV", bufs=1, space="PSUM"))
            psT = ctx.enter_context(tc.tile_pool(name="psT", bufs=2, space="PSUM"))

            # ---- stage A: x DMA spread over 3 hw queues, col-major order --
            xts = []
            for k in range(4):
                t = xtp.tile([128, N], BF16, tag=f"xt{k}", name=f"xt{k}")
                xts.append(t)
            quj = [nc.sync, nc.scalar]
            # first 512 columns only: the prologue QKV matmuls depend on the
            # per-queue DMA counter, so nothing else may be enqueued before
            # them (the dep is "queue counter >= N", not per-transfer)
            for k in range(4):
                quj[k % 2].dma_start(xts[k][:, 0:512], xt[k][:, 0:512])

            def emit_qk(j8, which):
                s_ = bass.ts(j8, 512)
                w, dst = (twq, qT) if which == "q" else (twk, kT)
                ps = psT.tile([128, 512], F32, tag="t", name="psqk")
                for k in range(4):
                    nc.tensor.matmul(
                        ps[:], w[:, bass.ts(k, 128)], xts[k][:, s_],
                        start=(k == 0), stop=(k == 3),
                    )
                nc.vector.tensor_copy(dst[:, s_], ps[:])

            def emit_v(jj):
                ps = psT.tile([128, 128], F32, tag="t", name="psv")
                for k in range(4):
                    nc.tensor.matmul(
                        ps[:], xts[k][:, bass.ts(jj, 128)], twv[:, bass.ts(k, 128)],
                        start=(k == 0), stop=(k == 3),
                    )
                dst = vaug[:, jj * VW : (jj + 1) * VW].rearrange(
                    "p (h c) -> p h c", h=2
                )[:, :, 0:D]
                src = ps[:].rearrange("p (h c) -> p h c", h=2)
                nc.vector.tensor_copy(dst, src)

            # (deadline in global i-steps, emitter) -- kT 128-col chunk
            # ci feeds scores at step ci (fine-grained for the first four);
            # kT 512-col group c feeds steps 4c..; v chunk jj feeds the AV
            # at step jj+2; q block j8 feeds step 32*j8.  Tasks are popped
            # AFTER the current step's scores, two steps ahead of need.
            def emit_kchunk(ci):
                ps = psT.tile([128, 128], F32, tag="t", name="pskc")
                for k in range(4):
                    nc.tensor.matmul(
                        ps[:], twk[:, bass.ts(k, 128)],
                        xts[k][:, bass.ts(ci, 128)],
                        start=(k == 0), stop=(k == 3),
                    )
                nc.vector.tensor_copy(kT[:, bass.ts(ci, 128)], ps[:])

            stage_a_tasks = []
            for ci in range(1, 4):
                stage_a_tasks.append((ci - 2, lambda ci=ci: emit_kchunk(ci)))
            for c in range(1, 8):
                stage_a_tasks.append((4 * c - 2, lambda c=c: emit_qk(c, "k")))
            for jj in range(NI):
                stage_a_tasks.append((jj, lambda jj=jj: emit_v(jj)))
            for j8 in range(1, 8):
                stage_a_tasks.append((32 * j8 - 6, lambda j8=j8: emit_qk(j8, "q")))
            stage_a_tasks.sort(key=lambda t: t[0])
            stage_a_tasks = list(stage_a_tasks)

            # prologue: exactly what scores(0) needs, nothing more
            emit_qk(0, "q")
            emit_kchunk(0)

            # rest of x and the projection weights arrive while the first
            # block runs (deferred so the prologue's queue-counter deps do
            # not cover them)
            # deferred transfers go on the gpsimd queue so the sync/scalar
            # queue counters the prologue matmuls wait on stay at exactly
            # the prefix transfers (a later enqueue on those queues widens
            # the wait and delays the first scores by several us)
            nc.gpsimd.dma_start(twv[:], wv[:])
            nc.gpsimd.dma_start(twp0[:], wp[0:64, :])
            nc.gpsimd.dma_start(twp1[:], wp[64:128, :])
            for k in range(4):
                nc.gpsimd.dma_start(xts[k][:, 512:2304], xt[k][:, 512:2304])
            for k in range(4):
                nc.gpsimd.dma_start(xts[k][:, 2304:4096], xt[k][:, 2304:4096])

            def emit_norm(st, after=None):
                # per-head: den staged to partition 0 (custom-DVE ops only
                # handle base-0 inputs), recip_approx_fast, bf16 cast,
                # K=1 bcast matmul, DVE multiply writes outT rows.
                phase, bk = st["phase"], st
                if phase == 0:
                    for h in range(2):
                        den = invp.tile([1, 512], F32, tag=f"den{h}", name="den")
                        (nc.vector if _NO_POOL else nc.gpsimd).tensor_copy(
                            den[:], bk["avsb"][h][64:65, :])
                        bk[f"den{h}"] = den
                elif phase == 1:
                    for h in range(2):
                        inv = invp.tile([1, 512], F32, tag=f"inv{h}", name="inv")
                        nc.vector.reciprocal_approx_fast(
                            inv[:], bk[f"den{h}"][:]
                        )
                        bk[f"inv{h}"] = inv
                elif phase == 2:
                    for h in range(2):
                        invb = invp.tile(
                            [1, 512], BF16, tag=f"invb{h}", name="invb"
                        )
                        (nc.vector if _NO_POOL else nc.gpsimd).tensor_copy(
                            invb[:], bk[f"inv{h}"][:])
                        bk[f"invb{h}"] = invb
                elif phase in (3, 4):
                    h = phase - 3
                    psb = psT.tile([64, 512], F32, tag="t", name="psb")
                    mi = nc.tensor.matmul(
                        psb[:], tones[0:1, 0:64], bk[f"invb{h}"][:],
                        start=True, stop=True,
                    )
                    if after is not None:
                        add_dep_helper(mi.ins, after.ins, sync=False,
                                       reason="tail behind scores")
                    qs = bk["qs"]
                    nc.vector.tensor_mul(
                        outTs[h][:, qs], bk["avsb"][h][0:64, :], psb[:]
                    )
                st["phase"] += 1

            def emit_proj(j, k, after=None, ot_scalar=False):
                jj = j * 4 + k
                s = bass.ts(jj, 128)
                pp = psT.tile([128, 512], F32, tag="t", name="pp")
                mi = nc.tensor.matmul(
                    pp[:], outT0[:, s], twp0[:], start=True, stop=False
                )
                nc.tensor.matmul(
                    pp[:], outT1[:, s], twp1[:], start=False, stop=True
                )
                if after is not None:
                    add_dep_helper(mi.ins, after.ins, sync=False,
                                   reason="tail behind scores")
                ot = outp.tile([128, 512], BF16, tag="o", name="ot")
                if ot_scalar:
                    nc.scalar.activation(ot[:], pp[:], AF.Copy)
                else:
                    nc.vector.tensor_copy(ot[:], pp[:])
                nc.sync.dma_start(out[s, :], ot[:])

            # ---- stage B/C flat pipeline over all 256 i-steps ----------
            # Per step: pending AV (2 steps behind, so the PE never waits
            # on exp), then scores+exp, then stage-A tasks and the previous
            # block's normalize/projection tail.
            prev = None     # tail state of the finished block
            pend = []       # AV emissions delayed 2 steps
            avs = None
            NT = 8 * NI
            for gs in range(NT + _AV_LAG):
                j, i = divmod(gs, NI)
                # scores + exp go FIRST so the ScalarE never waits behind
                # the AV pair in the in-order PE queue (ACT(gs) needs
                # scores(gs); the AV matmuls have 4 steps of slack).
                if gs < NT:
                    qs = bass.ts(j, 512)
                    ks = bass.ts(i, 128)
                    pss = psS.tile([128, 1024], F32, tag="s")
                    nc.tensor.matmul(
                        pss[:, 0:512], kT[0:64, ks], qT[0:64, qs],
                        start=True, stop=True,
                    )
                    last_sc = nc.tensor.matmul(
                        pss[:, 512:1024], kT[64:128, ks], qT[64:128, qs],
                        start=True, stop=True,
                    )
                    est = expp.tile([128, 1024], BF16, tag="e")
                    cs = _cs_for(gs)
                    nc.scalar.activation(
                        est[:, 0:cs], pss[:, 0:cs], AF.Exp, scale=SCALE
                    )
                    if cs < 1024:
                        nd = 1024 - cs
                        mid = midp.tile([128, 192], F32, tag="m", name="mid")
                        nc.vector._custom_dve(
                            EXP32_P1, out=mid[:, 0:nd], in0=pss[:, cs:1024],
                            s0=EXP_A, s1=EXP_C1, imm2=EXP_C3,
                        )
                        nc.vector._custom_dve(
                            EXP32_P2, out=est[:, cs:1024], in0=mid[:, 0:nd]
                        )
                # pending AVs (lag _AV_LAG); the stop-step's accumulators are
                # evacuated per head right behind their stop matmul.  Once
                # the score steps end (gs >= NT) flush everything at once --
                # draining one per step just stretches the epilogue.
                while pend and (pend[0][4] <= gs - _AV_LAG or gs >= NT):
                    p_avs, p_est, p_start, p_stop, p_gs, p_qs, p_j = pend.pop(0)
                    for h in range(2):
                        p_i = p_gs % NI
                        va = vaug[:, p_i * VW + h * 65 : p_i * VW + (h + 1) * 65]
                        nc.tensor.matmul(
                            p_avs[h][:], va, p_est[:, bass.ts(h, 512)],
                            start=p_start, stop=p_stop,
                        )
                        if p_stop and p_j == 7:
                            # last block: defer evacuation to the tail so it
                            # can interleave per 128-col chunk
                            if h == 1:
                                prev = {
                                    "j": p_j, "qs": p_qs, "phase": 0,
                                    "psav": p_avs, "avsb": None,
                                }
                        elif p_stop:
                            avsb = invp.tile(
                                [65, 512], F32, tag=f"avsb{h}", name="avsb"
                            )
                            nc.vector.tensor_copy(avsb[:], p_avs[h][:])
                            if h == 1:
                                prev = {
                                    "j": p_j, "qs": p_qs, "phase": 0,
                                    "avsb": [prev_avsb0, avsb],
                                }
                            else:
                                prev_avsb0 = avsb
                if gs < NT:
                    npop = 0
                    while (stage_a_tasks and stage_a_tasks[0][0] <= gs + 2
                           and npop < 3):
                        stage_a_tasks.pop(0)[1]()
                        npop += 1
                    # Norm phases start once the previous block's AV stop has
                    # been processed (python step i = _AV_LAG - 1 of this
                    # block); the slot pattern keeps one-step gaps so the
                    # DVE/Pool links of the chain pipeline.
                    _p0 = _AV_LAG - 1
                    _pj = max(13, _p0 + 8)
                    if prev is not None:
                        if i in (_p0, _p0 + 2, _p0 + 3, _p0 + 5, _p0 + 7):
                            emit_norm(prev, after=last_sc)
                        elif (i >= _pj and (i - _pj) % 4 == 0
                              and i <= _pj + 12):
                            emit_proj(prev["j"], (i - _pj) // 4, after=last_sc)
                    if i == 0:
                        avs = [
                            psAV.tile([65, 512], F32, tag=f"av{t}", name=f"av{t}")
                            for t in range(2)
                        ]
                    pend.append(
                        (avs, est, i == 0, i == NI - 1, gs, bass.ts(j, 512), j)
                    )
            # final block's tail: fine-grained 128-column pipeline (no next
            # block hides it, so shorten the critical chain instead)
            lj = prev["j"]
            avsbs = [
                invp.tile([65, 512], F32, tag=f"avsb{h}x", name="avsbx")
                for h in range(2)
            ]
            for k in range(2):
                ck = bass.ts(k, 256)
                cs_ = bass.ds(lj * 512 + k * 256, 256)
                for h in range(2):
                    # head 1's copy chain runs on the (now idle) ScalarE so
                    # the two heads pipeline on different engines instead of
                    # serializing on the DVE; 256-col chunks halve the
                    # semaphore hops vs 128-col ones
                    if h == 1:
                        nc.scalar.activation(
                            avsbs[h][:, ck], prev["psav"][h][:, ck], AF.Copy
                        )
                    else:
                        nc.vector.tensor_copy(
                            avsbs[h][:, ck], prev["psav"][h][:, ck]
                        )
                    denk = invp.tile([1, 256], F32, tag=f"denk{h}{k}",
                                     name="denk")
                    if h == 1:
                        nc.scalar.activation(
                            denk[:], avsbs[h][64:65, ck], AF.Copy
                        )
                    else:
                        nc.vector.tensor_copy(denk[:], avsbs[h][64:65, ck])
                    invk = invp.tile([1, 256], F32, tag=f"invk{h}{k}",
                                     name="invk")
                    nc.vector.reciprocal_approx_fast(invk[:], denk[:])
                    invkb = invp.tile([1, 256], BF16, tag=f"invkb{h}{k}",
                                      name="invkb")
                    if h == 1:
                        nc.scalar.activation(invkb[:], invk[:], AF.Copy)
                    else:
                        nc.vector.tensor_copy(invkb[:], invk[:])
                    psb = psT.tile([64, 256], F32, tag="t", name="psbk")
                    nc.tensor.matmul(
                        psb[:], tones[0:1, 0:64], invkb[:],
                        start=True, stop=True,
                    )
                    nc.vector.tensor_mul(
                        outTs[h][:, cs_], avsbs[h][0:64, ck], psb[:]
                    )
                emit_proj(lj, 2 * k, ot_scalar=True)
                emit_proj(lj, 2 * k + 1, ot_scalar=True)

    nc.compile()
    return nc


def _pack_w(wslice):
    # [512, 128] -> SBUF image [128, 4*128] with C-chunk k at cols k*128..
    return np.ascontiguousarray(
        wslice.reshape(4, 128, 128).transpose(1, 0, 2).reshape(128, 512)
    ).astype(BFNP)


_NC_CACHE = None
LAST_RESULT = None


def kernel(x, W_qkv, W_proj, b_proj):
    global _NC_CACHE, LAST_RESULT
    x = np.asarray(x, dtype=np.float32)
    W_qkv = np.asarray(W_qkv, dtype=np.float32)
    W_proj = np.asarray(W_proj, dtype=np.float32)
    b_proj = np.asarray(b_proj, dtype=np.float32)

    if _NC_CACHE is None:
        _NC_CACHE = build_nc()
    nc = _NC_CACHE

    in_maps = []
    for c in range(N_CORES):
        b = c // 4
        h0 = 2 * (c % 4)
        xtb = np.ascontiguousarray(x[b].T).reshape(4, 128, N).astype(BFNP)
        wq = _pack_w(W_qkv[:, h0 * 64 : h0 * 64 + 128])
        wk = _pack_w(W_qkv[:, 512 + h0 * 64 : 512 + h0 * 64 + 128])
        wv = _pack_w(W_qkv[:, 1024 + h0 * 64 : 1024 + h0 * 64 + 128])
        wp = np.ascontiguousarray(
            W_proj[h0 * 64 : (h0 + 2) * 64, :]
        ).astype(BFNP)
        in_maps.append({"xt": xtb, "wq": wq, "wk": wk, "wv": wv, "wp": wp})

    res = bass_utils.run_bass_kernel_spmd(
        nc, in_maps, core_ids=list(range(N_CORES))
    )
    LAST_RESULT = res

    out = np.zeros((B, N, C), dtype=np.float32)
    for c in range(N_CORES):
        out[c // 4] += res.results[c]["out"].astype(np.float32)
    out += b_proj[None, None, :]
    return out


# revision 34
# speedup vs baseline: 1.0552x; 1.0552x over previous
"""Multi-head attention (B=2, N=4096, C=512, H=8, d=64) on 8 Trainium2 NeuronCores.

Sharding: core c handles batch b = c//4 and heads {2*(c%4), 2*(c%4)+1}.
Each core computes its 2 heads' attention plus a partial output projection
(contraction over its 128 rows of W_proj); the host gather sums the 4
partials per batch and adds the bias.

On-device dataflow (transposed-scores formulation, no on-chip transposes):
  qT/kT [128=2*64 d-dims, 4096]  = W.T @ x.T      (x.T supplied by host)
  v_aug [128 n-chunk, 32*(65+65)] = x @ Wv with a ones column per head
  S^T[kidx, q] = kT.T_chunk @ qT  (two heads ride row-groups 0-1 / 2-3
                                   of the PE array concurrently, K=64 each)
  E = exp(S^T / 8)   -- ScalarE activation, one [128,1024] instruction per
                        step; this engine paces the steady state (1128ns/step).
                        A custom fused DVE exp (e^(s/8) = p(a*s)^32, two uOp
                        passes, ~3e-4 rel) exists behind NO_DVE_EXP=0/DVE_N,
                        but offloading columns to it consistently measures
                        ~40-70us SLOWER end to end (DVE-queue latency bursts
                        delay est -> AV stalls the PE, plus higher sustained
                        power triggers utilization throttling).
  [out_unnorm^T; den] = v_aug.T @ E   (ones column makes row 64 the softmax
                                       denominator -- no extra pass)
  out^T = out_unnorm^T * (1/den)  (reciprocal_approx_fast + K=2 broadcast
                                   matmul + DVE multiply)
  partial = out^T.T @ W_proj_slice   (K=128 packed contraction; AV runs two
                                      steps behind scores so the PE never
                                      waits on exp)
"""

import sys
import types

for _p in ("/opt/trn_rl_repo",):
    if _p not in sys.path:
        sys.path.insert(0, _p)

import numpy as np
import ml_dtypes
from contextlib import ExitStack

# antenv.axon_hooks shim: lets run_bass_kernel_spmd find the NTFF profiling
# hook when BASS_TRACE=1 (the agent image's antenv lacks this module).
import antenv  # noqa: F401

if "antenv.axon_hooks" not in sys.modules:
    _m = types.ModuleType("antenv.axon_hooks")
    _m._hook = None

    def _set_hook(h):
        _m._hook = h

    def _get_hook():
        return _m._hook

    _m.set_axon_ntff_profile_hook = _set_hook
    _m.get_axon_ntff_profile_hook = _get_hook
    sys.modules["antenv.axon_hooks"] = _m
    try:
        from trn_agent_boot.trn_boot import _ntff_profile_via_ctypes

        hook = _ntff_profile_via_ctypes("/opt/axon/libaxon_pjrt.so")
        if hook is not None:
            _set_hook(hook)
    except Exception:
        pass

import concourse.bass as bass  # noqa: E402
import concourse.tile as tile  # noqa: E402
from concourse.tile import add_dep_helper  # noqa: E402
from concourse import mybir, bacc  # noqa: E402
from concourse import bass_utils  # noqa: E402

# No bucket storage in this container; artifacts stay local.
bass_utils.upload_artifacts = lambda tmpdir: f"local://{tmpdir}"
# (walrus --enable-ldw-opt=true was tried and rejects this kernel:
# "InstLdweights is not compatible with LDW optimization")

# ---------------- custom DVE exp ops ----------------
from concourse import dve_ops  # noqa: E402
from concourse.dve_spec import Spec, Src0, C0, C1, C2, One, lower  # noqa: E402
from concourse.dve_spec import _has_src1 as has_src1  # noqa: E402
from concourse.dve_uop import DveOpSpec  # noqa: E402

EXP_A = 0.002765511489989489   # z = a*s; e^(s/8) = (1 + c1 z + z^2 + c3 z^3)^32
EXP_C1 = 1.4125294899368124
EXP_C3 = 0.46696463288916246


def _register_op(name, spec):
    for op in dve_ops.OPS:
        if op.name == name:
            return op
    row = dve_ops._CUSTOM_DVE_ROW_BASE + len(dve_ops.OPS)
    uops = lower(spec, ver="v3")
    sha = DveOpSpec(
        name=name, opcode=row, uops=uops, rd1_en=has_src1(spec)
    ).sha("v3")
    op = dve_ops.DveOp(name, spec, subdim=False, uops_sha={"v3": sha})
    dve_ops.OPS.append(op)
    dve_ops.CUSTOM_DVE_SPECS[name] = spec
    dve_ops._SUB_OPCODE_FOR_NAME[name] = row
    return op


def _p1_ref(in0, in1, s0, s1, imm2):
    z = (in0.astype(np.float32) * np.float32(s0)).astype(np.float32)
    q = (z * z).astype(np.float32)
    return (((np.float32(s1) + z) + np.float32(imm2) * q) * z).astype(np.float32)


def _p2_ref(in0, in1, s0, s1, imm2):
    p = in0.astype(np.float32) + np.float32(1.0)
    for _ in range(5):
        p = (p * p).astype(np.float32)
    return p


_z = Src0 * C0
_q = _z * _z
EXP32_P1 = _register_op(
    "ANT_EXP32_P1", Spec(body=((C1 + _z) + C2 * _q) * _z, reference=_p1_ref)
)
_p = Src0 + One
_r1 = _p * _p
_r2 = _r1 * _r1
_r3 = _r2 * _r2
_r4 = _r3 * _r3
EXP32_P2 = _register_op(
    "ANT_EXP32_P2", Spec(body=_r4 * _r4, reference=_p2_ref)
)

B, N, C = 2, 4096, 512
H, D = 8, 64
N_CORES = 8
SCALE = D ** -0.5

BF16 = mybir.dt.bfloat16
F32 = mybir.dt.float32
AF = mybir.ActivationFunctionType
BFNP = ml_dtypes.bfloat16

NI = N // 128   # 32 kidx / n chunks
VW = 2 * (D + 1)  # 130: per-n-chunk vaug block (2 heads x (64 v + 1 ones))


import os as _os

# ScalarE-only exp is the default: offloading exp columns to the DVE
# raises sustained power and triggers hardware utilization throttling
# that costs more than the offload saves (measured: blocks degrade from
# 1128ns/step to 1300+ with 60-90us of throttle-active time).
_NO_DVE_EXP = _os.environ.get("NO_DVE_EXP", "1") == "1"
_NO_POOL = _os.environ.get("NO_POOL", "0") == "1"


_DVE_N = int(_os.environ.get("DVE_N", "0"))
# AV lag 8 (vs 2): deeper decoupling of the AV accumulation from the exp
# producer absorbs ScalarE latency bursts so the ACT stream stays gapless;
# measured 298973ns (lag 8) vs 299762ns (lag 6) vs ~308000ns (lag 4) vs
# 333584ns (lag 2) -- same numerics, emission order only. Needs expp
# bufs >= lag + 2 and the lag-aware norm/proj slots below.
_AV_LAG = int(_os.environ.get("AV_LAG", "8"))


def _cs_for(gs):
    """ScalarE exp column count for step gs; DVE does [cs:1024]."""
    if _NO_DVE_EXP or _DVE_N == 0:
        return 1024
    if gs < 64:
        return max(928, 1024 - _DVE_N)
    return 1024 - _DVE_N


def build_nc():
    nc = bacc.Bacc("TRN2", target_bir_lowering=False, debug=False)

    xt = nc.dram_tensor("xt", [4, 128, N], BF16, kind="ExternalInput").ap()
    wq = nc.dram_tensor("wq", [128, 512], BF16, kind="ExternalInput").ap()
    wk = nc.dram_tensor("wk", [128, 512], BF16, kind="ExternalInput").ap()
    wv = nc.dram_tensor("wv", [128, 512], BF16, kind="ExternalInput").ap()
    wp = nc.dram_tensor("wp", [128, 512], BF16, kind="ExternalInput").ap()
    out = nc.dram_tensor("out", [N, C], BF16, kind="ExternalOutput").ap()

    with tile.TileContext(nc) as tc:
        with ExitStack() as ctx:
            const = ctx.enter_context(tc.tile_pool(name="const", bufs=1))
            sb = ctx.enter_context(tc.tile_pool(name="sb", bufs=1))
            expp = ctx.enter_context(tc.tile_pool(name="expp", bufs=10))
            midp = ctx.enter_context(tc.tile_pool(name="midp", bufs=3))
            invp = ctx.enter_context(tc.tile_pool(name="invp", bufs=2))
            outp = ctx.enter_context(tc.tile_pool(name="outp", bufs=3))

            # twq/twk ride the HW DMA queues (sync/scalar): the gpsimd
            # software DGE delivers them ~2.5us later (1300ns init + queue)
            # and the prologue's first matmul needs twq.
            twq = const.tile([128, 512], BF16)
            nc.sync.dma_start(twq[:], wq[:])
            twk = const.tile([128, 512], BF16)
            nc.scalar.dma_start(twk[:], wk[:])
            twv = const.tile([128, 512], BF16)
            twp0 = const.tile([64, 512], BF16)
            twp1 = const.tile([64, 512], BF16)
            tones = const.tile([1, 128], BF16)
            nc.gpsimd.memset(tones[:], 1.0)

            qT = sb.tile([128, N], BF16)
            kT = sb.tile([128, N], BF16)
            vaug = sb.tile([128, NI * VW], BF16)
            nc.gpsimd.memset(vaug[:], 1.0)
            outT0 = sb.tile([64, N], BF16)
            outT1 = sb.tile([64, N], BF16)
            outTs = (outT0, outT1)

            xtp = ctx.enter_context(tc.tile_pool(name="xtp", bufs=1))
            psS = ctx.enter_context(tc.tile_pool(name="psS", bufs=2, space="PSUM"))
            psAV = ctx.enter_context(tc.tile_pool(name="psAV", bufs=1, space="PSUM"))
            psT = ctx.enter_context(tc.tile_pool(name="psT", bufs=2, space="PSUM"))

            # ---- stage A: x DMA spread over 3 hw queues, col-major order --
            xts = []
            for k in range(4):
                t = xtp.tile([128, N], BF16, tag=f"xt{k}", name=f"xt{k}")
                xts.append(t)
            quj = [nc.sync, nc.scalar]
            # first 512 columns only: the prologue QKV matmuls depend on the
            # per-queue DMA counter, so nothing else may be enqueued before
            # them (the dep is "queue counter >= N", not per-transfer)
            for k in range(4):
                quj[k % 2].dma_start(xts[k][:, 0:512], xt[k][:, 0:512])

            def emit_qk(j8, which):
                s_ = bass.ts(j8, 512)
                w, dst = (twq, qT) if which == "q" else (twk, kT)
                ps = psT.tile([128, 512], F32, tag="t", name="psqk")
                for k in range(4):
                    nc.tensor.matmul(
                        ps[:], w[:, bass.ts(k, 128)], xts[k][:, s_],
                        start=(k == 0), stop=(k == 3),
                    )
                nc.vector.tensor_copy(dst[:, s_], ps[:])

            def emit_v(jj):
                ps = psT.tile([128, 128], F32, tag="t", name="psv")
                for k in range(4):
                    nc.tensor.matmul(
                        ps[:], xts[k][:, bass.ts(jj, 128)], twv[:, bass.ts(k, 128)],
                        start=(k == 0), stop=(k == 3),
                    )
                dst = vaug[:, jj * VW : (jj + 1) * VW].rearrange(
                    "p (h c) -> p h c", h=2
                )[:, :, 0:D]
                src = ps[:].rearrange("p (h c) -> p h c", h=2)
                nc.vector.tensor_copy(dst, src)

            # (deadline in global i-steps, emitter) -- kT 128-col chunk
            # ci feeds scores at step ci (fine-grained for the first four);
            # kT 512-col group c feeds steps 4c..; v chunk jj feeds the AV
            # at step jj+2; q block j8 feeds step 32*j8.  Tasks are popped
            # AFTER the current step's scores, two steps ahead of need.
            def emit_kchunk(ci):
                ps = psT.tile([128, 128], F32, tag="t", name="pskc")
                for k in range(4):
                    nc.tensor.matmul(
                        ps[:], twk[:, bass.ts(k, 128)],
                        xts[k][:, bass.ts(ci, 128)],
                        start=(k == 0), stop=(k == 3),
                    )
                nc.vector.tensor_copy(kT[:, bass.ts(ci, 128)], ps[:])

            stage_a_tasks = []
            for ci in range(1, 4):
                stage_a_tasks.append((ci - 2, lambda ci=ci: emit_kchunk(ci)))
            for c in range(1, 8):
                stage_a_tasks.append((4 * c - 2, lambda c=c: emit_qk(c, "k")))
            for jj in range(NI):
                stage_a_tasks.append((jj, lambda jj=jj: emit_v(jj)))
            for j8 in range(1, 8):
                stage_a_tasks.append((32 * j8 - 6, lambda j8=j8: emit_qk(j8, "q")))
            stage_a_tasks.sort(key=lambda t: t[0])
            stage_a_tasks = list(stage_a_tasks)

            # prologue: exactly what scores(0) needs, nothing more
            emit_qk(0, "q")
            emit_kchunk(0)

            # rest of x and the projection weights arrive while the first
            # block runs (deferred so the prologue's queue-counter deps do
            # not cover them)
            # deferred transfers go on the gpsimd queue so the sync/scalar
            # queue counters the prologue matmuls wait on stay at exactly
            # the prefix transfers (a later enqueue on those queues widens
            # the wait and delays the first scores by several us)
            nc.gpsimd.dma_start(twv[:], wv[:])
            nc.gpsimd.dma_start(twp0[:], wp[0:64, :])
            nc.gpsimd.dma_start(twp1[:], wp[64:128, :])
            for k in range(4):
                nc.gpsimd.dma_start(xts[k][:, 512:2304], xt[k][:, 512:2304])
            for k in range(4):
                nc.gpsimd.dma_start(xts[k][:, 2304:4096], xt[k][:, 2304:4096])

            def emit_norm(st, after=None):
                # per-head: den staged to partition 0 (custom-DVE ops only
                # handle base-0 inputs), recip_approx_fast, bf16 cast,
                # K=1 bcast matmul, DVE multiply writes outT rows.
                phase, bk = st["phase"], st
                if phase == 0:
                    for h in range(2):
                        den = invp.tile([1, 512], F32, tag=f"den{h}", name="den")
                        (nc.vector if _NO_POOL else nc.gpsimd).tensor_copy(
                            den[:], bk["avsb"][h][64:65, :])
                        bk[f"den{h}"] = den
                elif phase == 1:
                    for h in range(2):
                        inv = invp.tile([1, 512], F32, tag=f"inv{h}", name="inv")
                        nc.vector.reciprocal_approx_fast(
                            inv[:], bk[f"den{h}"][:]
                        )
                        bk[f"inv{h}"] = inv
                elif phase == 2:
                    for h in range(2):
                        invb = invp.tile(
                            [1, 512], BF16, tag=f"invb{h}", name="invb"
                        )
                        (nc.vector if _NO_POOL else nc.gpsimd).tensor_copy(
                            invb[:], bk[f"inv{h}"][:])
                        bk[f"invb{h}"] = invb
                elif phase in (3, 4):
                    h = phase - 3
                    psb = psT.tile([64, 512], F32, tag="t", name="psb")
                    mi = nc.tensor.matmul(
                        psb[:], tones[0:1, 0:64], bk[f"invb{h}"][:],
                        start=True, stop=True,
                    )
                    if after is not None:
                        add_dep_helper(mi.ins, after.ins, sync=False,
                                       reason="tail behind scores")
                    qs = bk["qs"]
                    nc.vector.tensor_mul(
                        outTs[h][:, qs], bk["avsb"][h][0:64, :], psb[:]
                    )
                st["phase"] += 1

            def emit_proj(j, k, after=None, ot_scalar=False):
                jj = j * 4 + k
                s = bass.ts(jj, 128)
                pp = psT.tile([128, 512], F32, tag="t", name="pp")
                mi = nc.tensor.matmul(
                    pp[:], outT0[:, s], twp0[:], start=True, stop=False
                )
                nc.tensor.matmul(
                    pp[:], outT1[:, s], twp1[:], start=False, stop=True
                )
                if after is not None:
                    add_dep_helper(mi.ins, after.ins, sync=False,
                                   reason="tail behind scores")
                ot = outp.tile([128, 512], BF16, tag="o", name="ot")
                if ot_scalar:
                    nc.scalar.activation(ot[:], pp[:], AF.Copy)
                else:
                    nc.vector.tensor_copy(ot[:], pp[:])
                nc.sync.dma_start(out[s, :], ot[:])

            # ---- stage B/C flat pipeline over all 256 i-steps ----------
            # Per step: pending AV (2 steps behind, so the PE never waits
            # on exp), then scores+exp, then stage-A tasks and the previous
            # block's normalize/projection tail.
            prev = None     # tail state of the finished block
            pend = []       # AV emissions delayed 2 steps
            avs = None
            NT = 8 * NI
            for gs in range(NT + _AV_LAG):
                j, i = divmod(gs, NI)
                # scores + exp go FIRST so the ScalarE never waits behind
                # the AV pair in the in-order PE queue (ACT(gs) needs
                # scores(gs); the AV matmuls have 4 steps of slack).
                if gs < NT:
                    qs = bass.ts(j, 512)
                    ks = bass.ts(i, 128)
                    pss = psS.tile([128, 1024], F32, tag="s")
                    nc.tensor.matmul(
                        pss[:, 0:512], kT[0:64, ks], qT[0:64, qs],
                        start=True, stop=True,
                    )
                    last_sc = nc.tensor.matmul(
                        pss[:, 512:1024], kT[64:128, ks], qT[64:128, qs],
                        start=True, stop=True,
                    )
                    est = expp.tile([128, 1024], BF16, tag="e")
                    cs = _cs_for(gs)
                    nc.scalar.activation(
                        est[:, 0:cs], pss[:, 0:cs], AF.Exp, scale=SCALE
                    )
                    if cs < 1024:
                        nd = 1024 - cs
                        mid = midp.tile([128, 192], F32, tag="m", name="mid")
                        nc.vector._custom_dve(
                            EXP32_P1, out=mid[:, 0:nd], in0=pss[:, cs:1024],
                            s0=EXP_A, s1=EXP_C1, imm2=EXP_C3,
                        )
                        nc.vector._custom_dve(
                            EXP32_P2, out=est[:, cs:1024], in0=mid[:, 0:nd]
                        )
                # pending AVs (lag _AV_LAG); the stop-step's accumulators are
                # evacuated per head right behind their stop matmul.  Once
                # the score steps end (gs >= NT) flush everything at once --
                # draining one per step just stretches the epilogue.
                while pend and (pend[0][4] <= gs - _AV_LAG or gs >= NT):
                    p_avs, p_est, p_start, p_stop, p_gs, p_qs, p_j = pend.pop(0)
                    for h in range(2):
                        p_i = p_gs % NI
                        va = vaug[:, p_i * VW + h * 65 : p_i * VW + (h + 1) * 65]
                        nc.tensor.matmul(
                            p_avs[h][:], va, p_est[:, bass.ts(h, 512)],
                            start=p_start, stop=p_stop,
                        )
                        if p_stop and p_j == 7:
                            # last block: defer evacuation to the tail so it
                            # can interleave per 128-col chunk
                            if h == 1:
                                prev = {
                                    "j": p_j, "qs": p_qs, "phase": 0,
                                    "psav": p_avs, "avsb": None,
                                }
                        elif p_stop:
                            avsb = invp.tile(
                                [65, 512], F32, tag=f"avsb{h}", name="avsb"
                            )
                            nc.vector.tensor_copy(avsb[:], p_avs[h][:])
                            if h == 1:
                                prev = {
                                    "j": p_j, "qs": p_qs, "phase": 0,
                                    "avsb": [prev_avsb0, avsb],
                                }
                            else:
                                prev_avsb0 = avsb
                if gs < NT:
                    npop = 0
                    while (stage_a_tasks and stage_a_tasks[0][0] <= gs + 2
                           and npop < 3):
                        stage_a_tasks.pop(0)[1]()
                        npop += 1
                    # Norm phases start once the previous block's AV stop has
                    # been processed (python step i = _AV_LAG - 1 of this
                    # block); the slot pattern keeps one-step gaps so the
                    # DVE/Pool links of the chain pipeline.
                    _p0 = _AV_LAG - 1
                    _pj = max(13, _p0 + 8)
                    if prev is not None:
                        if i in (_p0, _p0 + 2, _p0 + 3, _p0 + 5, _p0 + 7):
                            emit_norm(prev, after=last_sc)
                        elif (i >= _pj and (i - _pj) % 4 == 0
                              and i <= _pj + 12):
                            emit_proj(prev["j"], (i - _pj) // 4, after=last_sc)
                    if i == 0:
                        avs = [
                            psAV.tile([65, 512], F32, tag=f"av{t}", name=f"av{t}")
                            for t in range(2)
                        ]
                    pend.append(
                        (avs, est, i == 0, i == NI - 1, gs, bass.ts(j, 512), j)
                    )
            # final block's tail: fine-grained 128-column pipeline (no next
            # block hides it, so shorten the critical chain instead)
            lj = prev["j"]
            avsbs = [
                invp.tile([65, 512], F32, tag=f"avsb{h}x", name="avsbx")
                for h in range(2)
            ]
            for k in range(2):
                ck = bass.ts(k, 256)
                cs_ = bass.ds(lj * 512 + k * 256, 256)
                for h in range(2):
                    # head 1's copy chain runs on the (now idle) ScalarE so
                    # the two heads pipeline on different engines instead of
                    # serializing on the DVE; 256-col chunks halve the
                    # semaphore hops vs 128-col ones
                    if h == 1:
                        nc.scalar.activation(
                            avsbs[h][:, ck], prev["psav"][h][:, ck], AF.Copy
                        )
                    else:
                        nc.vector.tensor_copy(
                            avsbs[h][:, ck], prev["psav"][h][:, ck]
                        )
                    denk = invp.tile([1, 256], F32, tag=f"denk{h}{k}",
                                     name="denk")
                    if h == 1:
                        nc.scalar.activation(
                            denk[:], avsbs[h][64:65, ck], AF.Copy
                        )
                    else:
                        nc.vector.tensor_copy(denk[:], avsbs[h][64:65, ck])
                    invk = invp.tile([1, 256], F32, tag=f"invk{h}{k}",
                                     name="invk")
                    nc.vector.reciprocal_approx_fast(invk[:], denk[:])
                    invkb = invp.tile([1, 256], BF16, tag=f"invkb{h}{k}",
                                      name="invkb")
                    if h == 1:
                        nc.scalar.activation(invkb[:], invk[:], AF.Copy)
                    else:
                        nc.vector.tensor_copy(invkb[:], invk[:])
                    psb = psT.tile([64, 256], F32, tag="t", name="psbk")
                    nc.tensor.matmul(
                        psb[:], tones[0:1, 0:64], invkb[:],
                        start=True, stop=True,
                    )
                    nc.vector.tensor_mul(
                        outTs[h][:, cs_], avsbs[h][0:64, ck], psb[:]
                    )
                emit_proj(lj, 2 * k, ot_scalar=True)
                emit_proj(lj, 2 * k + 1, ot_scalar=True)

    nc.compile()
    return nc


def _pack_w(wslice):
    # [512, 128] -> SBUF image [128, 4*128] with C-chunk k at cols k*128..
    return np.ascontiguousarray(
        wslice.reshape(4, 128, 128).transpose(1, 0, 2).reshape(128, 512)
    ).astype(BFNP)


_NC_CACHE = None
LAST_RESULT = None


def kernel(x, W_qkv, W_proj, b_proj):
    global _NC_CACHE, LAST_RESULT
    x = np.asarray(x, dtype=np.float32)
    W_qkv = np.asarray(W_qkv, dtype=np.float32)
    W_proj = np.asarray(W_proj, dtype=np.float32)
    b_proj = np.asarray(b_proj, dtype=np.float32)

    if _NC_CACHE is None:
        _NC_CACHE = build_nc()
    nc = _NC_CACHE

    in_maps = []
    for c in range(N_CORES):
        b = c // 4
        h0 = 2 * (c % 4)
        xtb = np.ascontiguousarray(x[b].T).reshape(4, 128, N).astype(BFNP)
        wq = _pack_w(W_qkv[:, h0 * 64 : h0 * 64 + 128])
        wk = _pack_w(W_qkv[:, 512 + h0 * 64 : 512 + h0 * 64 + 128])
        wv = _pack_w(W_qkv[:, 1024 + h0 * 64 : 1024 + h0 * 64 + 128])
        wp = np.ascontiguousarray(
            W_proj[h0 * 64 : (h0 + 2) * 64, :]
        ).astype(BFNP)
        in_maps.append({"xt": xtb, "wq": wq, "wk": wk, "wv": wv, "wp": wp})

    res = bass_utils.run_bass_kernel_spmd(
        nc, in_maps, core_ids=list(range(N_CORES))
    )
    LAST_RESULT = res

    out = np.zeros((B, N, C), dtype=np.float32)
    for c in range(N_CORES):
        out[c // 4] += res.results[c]["out"].astype(np.float32)
    out += b_proj[None, None, :]
    return out
